# revision 19
# baseline (speedup 1.0000x reference)
"""Causal self-attention (B=4, T=2048, C=1024, H=16) on 8 trn2 NeuronCores.

Sharding: core c -> (batch b = c//2, head-group g = c%2 of 8 heads).
Each core computes qkv projection, causal attention and the proj partial-sum
for its 8 heads on its batch; the host sums the two head-group partials per
batch (row-parallel linear unshard).

Per-core kernel layout (all on-device matmuls bf16, f32 accumulation):
  xT [C, T] (pre-transposed on host) so QKV contraction runs with c on the
  partition axis with zero on-device transposes.
  QT/KT [2*64, T] per head pair -> scores S_T[t_k, t_q] via two k=64 matmuls
  packed into PE row-groups 0-63/64-127 (tile_position auto-derivation).
  Softmax without max-subtraction (logits ~ N(0,1), fp32-safe); denominator
  via an appended ones-column in the AV lhsT (m=65, row 64 = sum of exp).
  exp on ScalarE in [128, 2048] mega-tiles (bf16 PSUM) to amortize overhead.
  Normalization: reciprocal_approx_fast of denoms + GpSimd partition
  broadcast + one in-place multiply per pair; proj with k=128 chunks.
"""

from contextlib import ExitStack

import ml_dtypes
import numpy as np
import orjson

import concourse.bass as bass
import concourse.mybir as mybir
import concourse.tile as tile
from concourse.bass_utils import run_bass_kernel_spmd

BF16 = mybir.dt.bfloat16
F32 = mybir.dt.float32
F32R = mybir.dt.float32r
AF = mybir.ActivationFunctionType

T, C, H, DH = 2048, 1024, 16, 64
NCORES = 8
NPAIR = 4            # head pairs per core (8 heads)
CCH = C // 128       # contraction chunks for qkv
TQ = T // 512        # query chunks
NT = T // 128        # token tiles
VROW = 65            # 64 v-cols + ones column

# --- walrus in this env accepts only ONE sync-wait per instruction: split
# extras onto preceding same-engine NoOps at the BIR-JSON level.
if not getattr(bass.Bass, "_ant_wait_split", False):
    _orig_to_json_bytes = bass.Bass.to_json_bytes

    def _to_json_split_waits(self):
        m = orjson.loads(_orig_to_json_bytes(self))
        for f in m.get("functions", []):
            for bb in f.get("blocks") or []:
                insts = bb.get("instructions") or []
                out, changed = [], False
                for inst in insts:
                    si = inst.get("sync_info")
                    waits = (si or {}).get("on_wait") or []
                    if len(waits) > 1:
                        for j, w in enumerate(waits[:-1]):
                            out.append({
                                "debug": inst.get("debug", 0),
                                "engine": inst["engine"],
                                "ins": [], "outs": [],
                                "name": f"{inst['name']}-sw{j}",
                                "opcode": "NoOp",
                                "sync_info": {"on_wait": [w], "on_update": []},
                            })
                        si["on_wait"] = waits[-1:]
                        changed = True
                    out.append(inst)
                if changed:
                    bb["instructions"] = out
        return orjson.dumps(m)

    bass.Bass.to_json_bytes = _to_json_split_waits
    bass.Bass._ant_wait_split = True


def build_program() -> bass.Bass:
    nc = bass.Bass()
    xT = nc.dram_tensor("xT", [C, T], BF16, kind="ExternalInput")
    wqkvT = nc.dram_tensor("wqkvT", [C, 1536], BF16, kind="ExternalInput")
    wpT = nc.dram_tensor("wpT", [512, C], BF16, kind="ExternalInput")
    dmask = nc.dram_tensor("dmask", [128, 2048], BF16, kind="ExternalInput")
    seld = nc.dram_tensor("sel", [2, 128], F32R, kind="ExternalInput")
    out = nc.dram_tensor("out", [T, C], F32, kind="ExternalOutput")

    with ExitStack() as ctx:
        tc = ctx.enter_context(tile.TileContext(nc))
        const = ctx.enter_context(tc.tile_pool(name="const", bufs=1))
        pss = ctx.enter_context(tc.tile_pool(name="pss", bufs=2, space="PSUM"))
        psy = ctx.enter_context(tc.tile_pool(name="psy", bufs=2, space="PSUM"))
        ppool = ctx.enter_context(tc.tile_pool(name="ppool", bufs=2))
        spool = ctx.enter_context(tc.tile_pool(name="spool", bufs=2))
        rbpool = ctx.enter_context(tc.tile_pool(name="rbpool", bufs=2))
        opool = ctx.enter_context(tc.tile_pool(name="opool", bufs=2))
        dram = ctx.enter_context(tc.tile_pool(name="dram", bufs=1, space="DRAM"))
        dstage = dram.tile([16, 1024], F32, tag="dstage")
        rstage = dram.tile([16, 1024], F32, tag="rstage")

        xT_sb = const.tile([128, CCH, T], BF16, tag="xT")
        wq_sb = const.tile([128, CCH, 1536], BF16, tag="wq")
        wp_sb = const.tile([128, 4, C], BF16, tag="wp")
        dm_sb = const.tile([128, 2048], BF16, tag="dm")
        QT_sb = const.tile([128, NPAIR, T], BF16, tag="QT")
        KT_sb = const.tile([128, NPAIR, T], BF16, tag="KT")
        V_sb = const.tile([128, NT, 8 * VROW], BF16, tag="V")
        Yu_sb = const.tile([128, NPAIR, T], BF16, tag="Yu")

        for c in range(CCH):
            # split halves across DMA queues for a faster input ramp
            nc.sync.dma_start(xT_sb[:, c, 0:1024], xT[c * 128:(c + 1) * 128, 0:1024])
            nc.sync.dma_start(xT_sb[:, c, 1024:2048], xT[c * 128:(c + 1) * 128, 1024:2048])
            nc.sync.dma_start(wq_sb[:, c, 0:768], wqkvT[c * 128:(c + 1) * 128, 0:768])
            nc.sync.dma_start(wq_sb[:, c, 768:1536], wqkvT[c * 128:(c + 1) * 128, 768:1536])
        for c in range(4):
            nc.sync.dma_start(wp_sb[:, c, :], wpT[c * 128:(c + 1) * 128, :])
        nc.sync.dma_start(dm_sb[:], dmask[:])

        vr = V_sb[:].rearrange("p n (h e) -> p n h e", e=VROW)
        nc.gpsimd.memset(vr[:, :, :, 64:65], 1.0)
        # selector for the k=2 reciprocal-broadcast matmul:
        # out[m,:] = sel[0,m]*rt[0,:] + sel[1,m]*rt[1,:] -> A rows 0-63, B rows 64-127
        sel_sb = const.tile([128, 128], F32R, tag="sel")
        nc.sync.dma_start(sel_sb[0:2, :], seld[:])

        # ---------------- QKV projection ----------------
        for pair in range(NPAIR):
            for q in range(TQ):
                for colbase, dst in ((0, QT_sb), (512, KT_sb)):
                    ps = pss.tile([128, 512], F32, tag="ss")
                    for c in range(CCH):
                        nc.tensor.matmul(
                            ps[:],
                            wq_sb[:, c, colbase + pair * 128: colbase + (pair + 1) * 128],
                            xT_sb[:, c, q * 512:(q + 1) * 512],
                            start=(c == 0), stop=(c == CCH - 1),
                        )
                    nc.scalar.copy(dst[:, pair, q * 512:(q + 1) * 512], ps[:])
        for tt in range(NT):
            ps = pss.tile([128, 512], F32, tag="ss")
            for c in range(CCH):
                nc.tensor.matmul(
                    ps[:],
                    xT_sb[:, c, tt * 128:(tt + 1) * 128],
                    wq_sb[:, c, 1024:1536],
                    start=(c == 0), stop=(c == CCH - 1),
                )
            nc.scalar.copy(
                vr[:, tt, :, 0:64],
                ps[:].rearrange("p (h d) -> p h d", d=64),
            )

        # ---------------- attention ----------------
        for pair in range(NPAIR):
            hA, hB = 2 * pair, 2 * pair + 1
            for q in range(TQ):
                ya = psy.tile([VROW, 512], F32, tag="yA")
                yb = psy.tile([VROW, 512], F32, tag="yB")
                ntk = 4 * (q + 1)
                for tk in range(ntk):
                    ssm = pss.tile([128, 1024], F32, tag="ss")
                    nc.tensor.matmul(
                        ssm[:, 0:512],
                        KT_sb[0:64, pair, tk * 128:(tk + 1) * 128],
                        QT_sb[0:64, pair, q * 512:(q + 1) * 512],
                        start=True, stop=True,
                    )
                    nc.tensor.matmul(
                        ssm[:, 512:1024],
                        KT_sb[64:128, pair, tk * 128:(tk + 1) * 128],
                        QT_sb[64:128, pair, q * 512:(q + 1) * 512],
                        start=True, stop=True,
                    )
                    pm = ppool.tile([128, 1024], BF16, tag="P")
                    nc.scalar.activation(pm[:], ssm[:], AF.Exp)
                    if tk >= 4 * q:
                        off = (tk - 4 * q) * 512
                        nc.vector.tensor_mul(
                            pm[:, 0:512], pm[:, 0:512], dm_sb[:, off:off + 512])
                        nc.vector.tensor_mul(
                            pm[:, 512:1024], pm[:, 512:1024], dm_sb[:, off:off + 512])
                    first = (tk == 0)
                    last = (tk == ntk - 1)
                    nc.tensor.matmul(
                        ya[:], V_sb[:, tk, hA * VROW:(hA + 1) * VROW],
                        pm[:, 0:512],
                        start=first, stop=last,
                    )
                    nc.tensor.matmul(
                        yb[:], V_sb[:, tk, hB * VROW:(hB + 1) * VROW],
                        pm[:, 512:1024],
                        start=first, stop=last,
                    )
                # epilogue: strip denominators to DRAM staging, evac y
                dt = spool.tile([128, 1024], F32, tag="Dt")
                nc.vector.tensor_copy(dt[64:65, 0:512], ya[64:65, :])
                nc.vector.tensor_copy(dt[64:65, 512:1024], yb[64:65, :])
                nc.sync.dma_start(dstage[4 * pair + q, :], dt[64:65, :])
                nc.vector.tensor_copy(
                    Yu_sb[0:64, pair, q * 512:(q + 1) * 512], ya[0:64, :])
                bs = spool.tile([64, 512], BF16, tag="Bs")
                nc.vector.tensor_copy(bs[:], yb[0:64, :])
                nc.sync.dma_start(
                    Yu_sb[64:128, pair, q * 512:(q + 1) * 512], bs[:])
            # batched reciprocal of this pair's 4096 denominators ([128, 32])
            dp = spool.tile([128, 32], F32, tag="Dp")
            nc.sync.dma_start(
                dp[:],
                dstage[4 * pair:4 * pair + 4, :]
                .rearrange("q v -> (q v)").rearrange("(r c) -> r c", c=32))
            rp = spool.tile([128, 32], F32, tag="Rp")
            nc.vector.reciprocal(rp[:], dp[:])
            nc.sync.dma_start(
                rstage[4 * pair:4 * pair + 4, :]
                .rearrange("q v -> (q v)").rearrange("(r c) -> r c", c=32),
                rp[:])
        # normalize: k=2 PE broadcast of reciprocals + one in-place mul per pair
        # (kept out of the attention loop so the DMA/recip chain never blocks
        # the in-order PE stream between pairs)
        for pair in range(NPAIR):
            RB = rbpool.tile([128, T], F32, tag="RB")
            for q in range(TQ):
                rt = spool.tile([128, 512], F32R, tag="Rt")
                nc.sync.dma_start(
                    rt[0:2, :],
                    rstage[4 * pair + q, :]
                    .rearrange("(a c) -> a c", c=512).bitcast(F32R))
                bc = pss.tile([128, 512], F32, tag="ss")
                nc.tensor.matmul(
                    bc[:], sel_sb[0:2, :], rt[0:2, :], start=True, stop=True)
                nc.vector.tensor_copy(RB[:, q * 512:(q + 1) * 512], bc[:])
            nc.vector.tensor_mul(Yu_sb[:, pair, :], Yu_sb[:, pair, :], RB[:])

        # ---------------- output projection (partial over this core's heads) --
        for tt in range(NT):
            ot = opool.tile([128, C], F32, tag="Ot")
            for oc in range(2):
                po = pss.tile([128, 512], F32, tag="ss")
                for pair in range(NPAIR):
                    nc.tensor.matmul(
                        po[:],
                        Yu_sb[:, pair, tt * 128:(tt + 1) * 128],
                        wp_sb[:, pair, oc * 512:(oc + 1) * 512],
                        start=(pair == 0), stop=(pair == NPAIR - 1),
                    )
                nc.vector.tensor_copy(ot[:, oc * 512:(oc + 1) * 512], po[:])
            nc.sync.dma_start(out[tt * 128:(tt + 1) * 128, :], ot[:])

    return nc


def make_in_maps(x: np.ndarray, w_qkv: np.ndarray, w_proj: np.ndarray):
    bf = ml_dtypes.bfloat16
    scale = np.float32(DH ** -0.5)

    iq = np.arange(512)[None, :]
    ik = np.arange(128)[:, None]
    dmask = np.concatenate(
        [(iq >= j * 128 + ik) for j in range(4)], axis=1).astype(bf)

    in_maps = []
    for core in range(NCORES):
        b, g = core // 2, core % 2
        xTb = np.ascontiguousarray(x[b].T).astype(bf)           # [C, T]
        wq = (w_qkv[512 * g: 512 * g + 512] * scale).astype(np.float32)
        wk = w_qkv[1024 + 512 * g: 1024 + 512 * g + 512]
        wv = w_qkv[2048 + 512 * g: 2048 + 512 * g + 512]
        wqkvT = np.ascontiguousarray(
            np.concatenate([wq, wk, wv], axis=0).T).astype(bf)  # [C, 1536]
        wpT = np.ascontiguousarray(
            w_proj[:, 512 * g: 512 * g + 512].T).astype(bf)     # [512, C]
        sel = np.zeros((2, 128), dtype=np.float32)
        sel[0, 0:64] = 1.0
        sel[1, 64:128] = 1.0
        in_maps.append({"xT": xTb, "wqkvT": wqkvT, "wpT": wpT, "dmask": dmask,
                        "sel": sel})
    return in_maps


_NC = None


def kernel(x: np.ndarray, w_qkv: np.ndarray, w_proj: np.ndarray,
           _trace: bool = False, _return_raw: bool = False) -> np.ndarray:
    global _NC
    x = np.asarray(x, dtype=np.float32)
    w_qkv = np.asarray(w_qkv, dtype=np.float32)
    w_proj = np.asarray(w_proj, dtype=np.float32)
    if _NC is None:
        _NC = build_program()
    in_maps = make_in_maps(x, w_qkv, w_proj)
    res = run_bass_kernel_spmd(_NC, in_maps, list(range(NCORES)), trace=_trace)
    B = x.shape[0]
    outp = np.empty((B, T, C), dtype=np.float32)
    for b in range(B):
        outp[b] = res.results[2 * b]["out"] + res.results[2 * b + 1]["out"]
    if _return_raw:
        return outp, res
    return outp


# revision 20
# speedup vs baseline: 1.0242x; 1.0242x over previous
"""Causal self-attention (B=4, T=2048, C=1024, H=16) on 8 trn2 NeuronCores.

Sharding: core c -> (batch b = c//2, head-group g = c%2 of 8 heads).
Each core computes qkv projection, causal attention and the proj partial-sum
for its 8 heads on its batch; the host sums the two head-group partials per
batch (row-parallel linear unshard).

Per-core kernel layout (all on-device matmuls bf16, f32 accumulation):
  xT [C, T] (pre-transposed on host) so QKV contraction runs with c on the
  partition axis with zero on-device transposes.
  QT/KT [2*64, T] per head pair -> scores S_T[t_k, t_q] via two k=64 matmuls
  packed into PE row-groups 0-63/64-127 (tile_position auto-derivation).
  Softmax without max-subtraction (logits ~ N(0,1), fp32-safe); denominator
  via an appended ones-column in the AV lhsT (m=65, row 64 = sum of exp).
  exp on ScalarE in [128, 2048] mega-tiles (bf16 PSUM) to amortize overhead.
  Normalization: reciprocal_approx_fast of denoms + GpSimd partition
  broadcast + one in-place multiply per pair; proj with k=128 chunks.
"""

from contextlib import ExitStack

import ml_dtypes
import numpy as np
import orjson

import concourse.bass as bass
import concourse.mybir as mybir
import concourse.tile as tile
from concourse.bass_utils import run_bass_kernel_spmd

BF16 = mybir.dt.bfloat16
F32 = mybir.dt.float32
F32R = mybir.dt.float32r
AF = mybir.ActivationFunctionType

T, C, H, DH = 2048, 1024, 16, 64
NCORES = 8
NPAIR = 4            # head pairs per core (8 heads)
CCH = C // 128       # contraction chunks for qkv
TQ = T // 512        # query chunks
NT = T // 128        # token tiles
VROW = 65            # 64 v-cols + ones column

# --- walrus in this env accepts only ONE sync-wait per instruction: split
# extras onto preceding same-engine NoOps at the BIR-JSON level.
if not getattr(bass.Bass, "_ant_wait_split", False):
    _orig_to_json_bytes = bass.Bass.to_json_bytes

    def _to_json_split_waits(self):
        m = orjson.loads(_orig_to_json_bytes(self))
        for f in m.get("functions", []):
            for bb in f.get("blocks") or []:
                insts = bb.get("instructions") or []
                out, changed = [], False
                for inst in insts:
                    si = inst.get("sync_info")
                    waits = (si or {}).get("on_wait") or []
                    if len(waits) > 1:
                        for j, w in enumerate(waits[:-1]):
                            out.append({
                                "debug": inst.get("debug", 0),
                                "engine": inst["engine"],
                                "ins": [], "outs": [],
                                "name": f"{inst['name']}-sw{j}",
                                "opcode": "NoOp",
                                "sync_info": {"on_wait": [w], "on_update": []},
                            })
                        si["on_wait"] = waits[-1:]
                        changed = True
                    out.append(inst)
                if changed:
                    bb["instructions"] = out
        return orjson.dumps(m)

    bass.Bass.to_json_bytes = _to_json_split_waits
    bass.Bass._ant_wait_split = True


def build_program() -> bass.Bass:
    nc = bass.Bass()
    xT = nc.dram_tensor("xT", [C, T], BF16, kind="ExternalInput")
    wqkvT = nc.dram_tensor("wqkvT", [C, 1536], BF16, kind="ExternalInput")
    wpT = nc.dram_tensor("wpT", [512, C], BF16, kind="ExternalInput")
    dmask = nc.dram_tensor("dmask", [128, 2048], BF16, kind="ExternalInput")
    seld = nc.dram_tensor("sel", [2, 128], F32R, kind="ExternalInput")
    out = nc.dram_tensor("out", [T, C], F32, kind="ExternalOutput")

    with ExitStack() as ctx:
        tc = ctx.enter_context(tile.TileContext(nc))
        const = ctx.enter_context(tc.tile_pool(name="const", bufs=1))
        pss = ctx.enter_context(tc.tile_pool(name="pss", bufs=2, space="PSUM"))
        psy = ctx.enter_context(tc.tile_pool(name="psy", bufs=2, space="PSUM"))
        ppool = ctx.enter_context(tc.tile_pool(name="ppool", bufs=3))
        spool = ctx.enter_context(tc.tile_pool(name="spool", bufs=2))
        rbpool = ctx.enter_context(tc.tile_pool(name="rbpool", bufs=2))
        opool = ctx.enter_context(tc.tile_pool(name="opool", bufs=2))
        dram = ctx.enter_context(tc.tile_pool(name="dram", bufs=1, space="DRAM"))
        dstage = dram.tile([16, 1024], F32, tag="dstage")
        rstage = dram.tile([16, 1024], F32, tag="rstage")

        xT_sb = const.tile([128, CCH, T], BF16, tag="xT")
        wq_sb = const.tile([128, CCH, 1536], BF16, tag="wq")
        wp_sb = const.tile([128, 4, C], BF16, tag="wp")
        dm_sb = const.tile([128, 2048], BF16, tag="dm")
        QT_sb = const.tile([128, NPAIR, T], BF16, tag="QT")
        KT_sb = const.tile([128, NPAIR, T], BF16, tag="KT")
        V_sb = const.tile([128, NT, 8 * VROW], BF16, tag="V")
        Yu_sb = const.tile([128, NPAIR, T], BF16, tag="Yu")

        for c in range(CCH):
            # split halves across DMA queues for a faster input ramp
            nc.sync.dma_start(xT_sb[:, c, 0:1024], xT[c * 128:(c + 1) * 128, 0:1024])
            nc.sync.dma_start(xT_sb[:, c, 1024:2048], xT[c * 128:(c + 1) * 128, 1024:2048])
            nc.sync.dma_start(wq_sb[:, c, 0:768], wqkvT[c * 128:(c + 1) * 128, 0:768])
            nc.sync.dma_start(wq_sb[:, c, 768:1536], wqkvT[c * 128:(c + 1) * 128, 768:1536])
        for c in range(4):
            nc.sync.dma_start(wp_sb[:, c, :], wpT[c * 128:(c + 1) * 128, :])
        nc.sync.dma_start(dm_sb[:], dmask[:])

        vr = V_sb[:].rearrange("p n (h e) -> p n h e", e=VROW)
        nc.gpsimd.memset(vr[:, :, :, 64:65], 1.0)
        # selector for the k=2 reciprocal-broadcast matmul:
        # out[m,:] = sel[0,m]*rt[0,:] + sel[1,m]*rt[1,:] -> A rows 0-63, B rows 64-127
        sel_sb = const.tile([128, 128], F32R, tag="sel")
        nc.sync.dma_start(sel_sb[0:2, :], seld[:])

        # ---------------- QKV projection ----------------
        for pair in range(NPAIR):
            for q in range(TQ):
                for colbase, dst in ((0, QT_sb), (512, KT_sb)):
                    ps = pss.tile([128, 512], F32, tag="ss")
                    for c in range(CCH):
                        nc.tensor.matmul(
                            ps[:],
                            wq_sb[:, c, colbase + pair * 128: colbase + (pair + 1) * 128],
                            xT_sb[:, c, q * 512:(q + 1) * 512],
                            start=(c == 0), stop=(c == CCH - 1),
                        )
                    nc.scalar.copy(dst[:, pair, q * 512:(q + 1) * 512], ps[:])
        for tt in range(NT):
            ps = pss.tile([128, 512], F32, tag="ss")
            for c in range(CCH):
                nc.tensor.matmul(
                    ps[:],
                    xT_sb[:, c, tt * 128:(tt + 1) * 128],
                    wq_sb[:, c, 1024:1536],
                    start=(c == 0), stop=(c == CCH - 1),
                )
            nc.scalar.copy(
                vr[:, tt, :, 0:64],
                ps[:].rearrange("p (h d) -> p h d", d=64),
            )

        # ---------------- attention ----------------
        for pair in range(NPAIR):
            hA, hB = 2 * pair, 2 * pair + 1
            for q in range(TQ):
                ya = psy.tile([VROW, 512], F32, tag="yA")
                yb = psy.tile([VROW, 512], F32, tag="yB")
                ntk = 4 * (q + 1)
                for tk in range(ntk):
                    ssm = pss.tile([128, 1024], F32, tag="ss")
                    nc.tensor.matmul(
                        ssm[:, 0:512],
                        KT_sb[0:64, pair, tk * 128:(tk + 1) * 128],
                        QT_sb[0:64, pair, q * 512:(q + 1) * 512],
                        start=True, stop=True,
                    )
                    nc.tensor.matmul(
                        ssm[:, 512:1024],
                        KT_sb[64:128, pair, tk * 128:(tk + 1) * 128],
                        QT_sb[64:128, pair, q * 512:(q + 1) * 512],
                        start=True, stop=True,
                    )
                    pm = ppool.tile([128, 1024], BF16, tag="P")
                    nc.scalar.activation(pm[:], ssm[:], AF.Exp)
                    if tk >= 4 * q:
                        off = (tk - 4 * q) * 512
                        nc.vector.tensor_mul(
                            pm[:, 0:512], pm[:, 0:512], dm_sb[:, off:off + 512])
                        nc.vector.tensor_mul(
                            pm[:, 512:1024], pm[:, 512:1024], dm_sb[:, off:off + 512])
                    first = (tk == 0)
                    last = (tk == ntk - 1)
                    nc.tensor.matmul(
                        ya[:], V_sb[:, tk, hA * VROW:(hA + 1) * VROW],
                        pm[:, 0:512],
                        start=first, stop=last,
                    )
                    nc.tensor.matmul(
                        yb[:], V_sb[:, tk, hB * VROW:(hB + 1) * VROW],
                        pm[:, 512:1024],
                        start=first, stop=last,
                    )
                # epilogue: strip denominators to DRAM staging, evac y
                dt = spool.tile([128, 1024], F32, tag="Dt")
                nc.vector.tensor_copy(dt[64:65, 0:512], ya[64:65, :])
                nc.vector.tensor_copy(dt[64:65, 512:1024], yb[64:65, :])
                nc.sync.dma_start(dstage[4 * pair + q, :], dt[64:65, :])
                nc.vector.tensor_copy(
                    Yu_sb[0:64, pair, q * 512:(q + 1) * 512], ya[0:64, :])
                bs = spool.tile([64, 512], BF16, tag="Bs")
                nc.vector.tensor_copy(bs[:], yb[0:64, :])
                nc.sync.dma_start(
                    Yu_sb[64:128, pair, q * 512:(q + 1) * 512], bs[:])
            # batched reciprocal of this pair's 4096 denominators ([128, 32])
            dp = spool.tile([128, 32], F32, tag="Dp")
            nc.sync.dma_start(
                dp[:],
                dstage[4 * pair:4 * pair + 4, :]
                .rearrange("q v -> (q v)").rearrange("(r c) -> r c", c=32))
            rp = spool.tile([128, 32], F32, tag="Rp")
            nc.vector.reciprocal(rp[:], dp[:])
            nc.sync.dma_start(
                rstage[4 * pair:4 * pair + 4, :]
                .rearrange("q v -> (q v)").rearrange("(r c) -> r c", c=32),
                rp[:])
        # normalize: k=2 PE broadcast of reciprocals + one in-place mul per pair
        # (kept out of the attention loop so the DMA/recip chain never blocks
        # the in-order PE stream between pairs)
        for pair in range(NPAIR):
            RB = rbpool.tile([128, T], F32, tag="RB")
            for q in range(TQ):
                rt = spool.tile([128, 512], F32R, tag="Rt")
                nc.sync.dma_start(
                    rt[0:2, :],
                    rstage[4 * pair + q, :]
                    .rearrange("(a c) -> a c", c=512).bitcast(F32R))
                bc = pss.tile([128, 512], F32, tag="ss")
                nc.tensor.matmul(
                    bc[:], sel_sb[0:2, :], rt[0:2, :], start=True, stop=True)
                nc.vector.tensor_copy(RB[:, q * 512:(q + 1) * 512], bc[:])
            nc.vector.tensor_mul(Yu_sb[:, pair, :], Yu_sb[:, pair, :], RB[:])

        # ---------------- output projection (partial over this core's heads) --
        for tt in range(NT):
            ot = opool.tile([128, C], F32, tag="Ot")
            for oc in range(2):
                po = pss.tile([128, 512], F32, tag="ss")
                for pair in range(NPAIR):
                    nc.tensor.matmul(
                        po[:],
                        Yu_sb[:, pair, tt * 128:(tt + 1) * 128],
                        wp_sb[:, pair, oc * 512:(oc + 1) * 512],
                        start=(pair == 0), stop=(pair == NPAIR - 1),
                    )
                nc.vector.tensor_copy(ot[:, oc * 512:(oc + 1) * 512], po[:])
            nc.sync.dma_start(out[tt * 128:(tt + 1) * 128, :], ot[:])

    return nc


def make_in_maps(x: np.ndarray, w_qkv: np.ndarray, w_proj: np.ndarray):
    bf = ml_dtypes.bfloat16
    scale = np.float32(DH ** -0.5)

    iq = np.arange(512)[None, :]
    ik = np.arange(128)[:, None]
    dmask = np.concatenate(
        [(iq >= j * 128 + ik) for j in range(4)], axis=1).astype(bf)

    in_maps = []
    for core in range(NCORES):
        b, g = core // 2, core % 2
        xTb = np.ascontiguousarray(x[b].T).astype(bf)           # [C, T]
        wq = (w_qkv[512 * g: 512 * g + 512] * scale).astype(np.float32)
        wk = w_qkv[1024 + 512 * g: 1024 + 512 * g + 512]
        wv = w_qkv[2048 + 512 * g: 2048 + 512 * g + 512]
        wqkvT = np.ascontiguousarray(
            np.concatenate([wq, wk, wv], axis=0).T).astype(bf)  # [C, 1536]
        wpT = np.ascontiguousarray(
            w_proj[:, 512 * g: 512 * g + 512].T).astype(bf)     # [512, C]
        sel = np.zeros((2, 128), dtype=np.float32)
        sel[0, 0:64] = 1.0
        sel[1, 64:128] = 1.0
        in_maps.append({"xT": xTb, "wqkvT": wqkvT, "wpT": wpT, "dmask": dmask,
                        "sel": sel})
    return in_maps


_NC = None


def kernel(x: np.ndarray, w_qkv: np.ndarray, w_proj: np.ndarray,
           _trace: bool = False, _return_raw: bool = False) -> np.ndarray:
    global _NC
    x = np.asarray(x, dtype=np.float32)
    w_qkv = np.asarray(w_qkv, dtype=np.float32)
    w_proj = np.asarray(w_proj, dtype=np.float32)
    if _NC is None:
        _NC = build_program()
    in_maps = make_in_maps(x, w_qkv, w_proj)
    res = run_bass_kernel_spmd(_NC, in_maps, list(range(NCORES)), trace=_trace)
    B = x.shape[0]
    outp = np.empty((B, T, C), dtype=np.float32)
    for b in range(B):
        outp[b] = res.results[2 * b]["out"] + res.results[2 * b + 1]["out"]
    if _return_raw:
        return outp, res
    return outp
